# revision 1
# baseline (speedup 1.0000x reference)
"""Trainium2 Bass kernel for nn_AttentionBlock (masked GroupNorm + jagged full attention).

Contract: kernel(**inputs) takes FULL unsharded inputs (as in reference.setup_inputs())
and returns the FULL [8, 1024, 512] fp32 output. Internally shards data-parallel over
the batch: sample b -> NeuronCore b (8 cores).

Per-core dataflow (sample s, all layouts chosen so every matmul contracts over the
partition dim with no on-device transposes):
  xmT [C,L] (host-masked, transposed)
    -> GroupNorm stats via bn_stats + group-selector matmuls -> xnT bf16 [C,L]
    -> qkT = Wqkv[:, :1024].T @ xn   (transposed q/k layout [64*dh, L] per head pair)
    -> v   = xn @ Wqkv[:, 1024:]    (natural layout, masked rows, + softmax mask-row)
    -> scoresT[k,q] per head (pairs packed in the PE array via 64-row tiles)
    -> exp on ScalarE (scale=1/8 folded; masking not needed: V rows of padded
       tokens are zeroed and the appended mask-row yields masked denominators)
    -> attn-out^T = [v | maskrow].T @ expT  (row 64 = softmax denominator)
    -> denominators -> reciprocal -> broadcast matmul -> normalize (padded q
       columns zeroed via +inf denominators)
    -> proj: out[q,c] = attnT.T @ Wproj, + host-prepared residual (x*mask with all
       bias terms folded in), DMA out.
"""

import numpy as np
import ml_dtypes
from contextlib import ExitStack

B, L, C, G, H = 8, 1024, 512, 32, 8
DH = C // H          # 64
CPG = C // G         # 16
EPS = 1e-5
NT = L // 128        # 8 token tiles
CT = C // 128        # 4 channel tiles
QC = L // 512        # 2 query chunks

BF16 = ml_dtypes.bfloat16

_CACHE = {}


def _build():
    import concourse.bass as bass
    import concourse.tile as tile
    from concourse import bacc, mybir

    f32 = mybir.dt.float32
    f16 = mybir.dt.float16
    bf16 = mybir.dt.bfloat16
    Alu = mybir.AluOpType
    Act = mybir.ActivationFunctionType

    nc = bacc.Bacc("TRN2", target_bir_lowering=False)

    # ---- per-core DRAM inputs (host-prepped) ----
    xmT_d = nc.dram_tensor("xmT", [C, L], f32, kind="ExternalInput")
    xm_d = nc.dram_tensor("xmr", [L, C], f32, kind="ExternalInput")    # residual: x*mf + folded biases
    wqk_d = nc.dram_tensor("wqk", [CT, 8, 128, 128], bf16, kind="ExternalInput")  # lhsT tiles (q,k couts)
    wv_d = nc.dram_tensor("wv", [CT, 128, C], bf16, kind="ExternalInput")         # rhs tiles (v couts)
    wp_d = nc.dram_tensor("wp", [CT, 128, C], bf16, kind="ExternalInput")         # Wproj rhs tiles
    bqk_d = nc.dram_tensor("bqk", [128, 8], f32, kind="ExternalInput")            # qk bias per couttile
    gam_d = nc.dram_tensor("gam", [128, CT], f32, kind="ExternalInput")           # gamma per cintile
    vmask_d = nc.dram_tensor("vmask", [128, NT], f32, kind="ExternalInput")       # token mask per ktile
    qmv_d = nc.dram_tensor("qmv", [1, L], f32, kind="ExternalInput")               # 1 valid / 1e30 pad
    sel_d = nc.dram_tensor("sel", [CT, 128, G], f32, kind="ExternalInput")        # chan->group selector
    selT_d = nc.dram_tensor("selT", [G, C], f32, kind="ExternalInput")            # group->chan selector
    icnt_d = nc.dram_tensor("icnt", [G, 1], f32, kind="ExternalInput")            # 1/(len*cpg)
    out_d = nc.dram_tensor("out", [L, C], f32, kind="ExternalOutput")

    with tile.TileContext(nc) as tc, ExitStack() as ctx:
        pc = ctx.enter_context(tc.tile_pool(name="consts", bufs=1))
        pb = ctx.enter_context(tc.tile_pool(name="big", bufs=1))
        ps = ctx.enter_context(tc.tile_pool(name="psum", bufs=1, space="PSUM"))

        # ---- constant / input loads (small consts first: they gate GroupNorm) ----
        bqk_sb = pc.tile([128, 8], f32, tag="bqk", name="bqk")
        gam_sb = pc.tile([128, CT], f32, tag="gam", name="gam")
        vmask_sb = pc.tile([128, NT], f32, tag="vmask", name="vmask")
        qmv_sb = pc.tile([1, L], f32, tag="qmv", name="qmv")
        selT_sb = pc.tile([G, C], f32, tag="selT", name="selT")
        icnt_sb = pc.tile([G, 1], f32, tag="icnt", name="icnt")
        for t_sb, t_d in [(bqk_sb, bqk_d), (gam_sb, gam_d), (vmask_sb, vmask_d),
                          (qmv_sb, qmv_d), (selT_sb, selT_d), (icnt_sb, icnt_d)]:
            nc.sync.dma_start(t_sb[:], t_d[:, :])
        sel_sb = [pc.tile([128, G], f32, tag=f"sel{ct}", name=f"sel{ct}") for ct in range(CT)]
        for ct in range(CT):
            nc.sync.dma_start(sel_sb[ct][:], sel_d[ct])
        xmt_sb = [pb.tile([128, L], f32, tag=f"xmT{t}", name=f"xmT{t}") for t in range(CT)]
        for t in range(CT):
            nc.sync.dma_start(xmt_sb[t][:], xmT_d[128 * t:128 * (t + 1), :])
        wqk_sb = [[pc.tile([128, 128], bf16, tag=f"wqk{ct}_{ot}", name=f"wqk{ct}_{ot}") for ot in range(8)] for ct in range(CT)]
        for ct in range(CT):
            for ot in range(8):
                nc.sync.dma_start(wqk_sb[ct][ot][:], wqk_d[ct, ot])
        wv_sb = [pc.tile([128, C], bf16, tag=f"wv{ct}", name=f"wv{ct}") for ct in range(CT)]
        wp_sb = [pc.tile([128, C], bf16, tag=f"wp{ct}", name=f"wp{ct}") for ct in range(CT)]
        for ct in range(CT):
            nc.sync.dma_start(wv_sb[ct][:], wv_d[ct])
            nc.sync.dma_start(wp_sb[ct][:], wp_d[ct])
        # ---- Phase 1: GroupNorm (stats over valid tokens; zeros from host masking) ----
        smm = [pb.tile([128, 2], f32, tag=f"smm{t}", name=f"smm{t}") for t in range(CT)]
        ps_g = ps.tile([G, 2], f32, tag="avA", name="avA", bufs=2)
        for t in range(CT):
            bns = pb.tile([128, 2, 6], f32, tag="bns", name="bns")
            nc.vector.bn_stats(bns[:, 0, :], xmt_sb[t][:, 0:512])
            nc.vector.bn_stats(bns[:, 1, :], xmt_sb[t][:, 512:1024])
            mv = pb.tile([128, 2], f32, tag="mv", name="mv")
            nc.vector.bn_aggr(mv[:], bns[:])
            sq = pb.tile([128, 1], f32, tag="sq", name="sq")
            nc.vector.tensor_mul(sq[:], mv[:, 0:1], mv[:, 0:1])
            # smm = [sum(x), sum(x^2)] recovered from mean/var over all 1024 (incl. zeros)
            nc.vector.tensor_scalar(smm[t][:, 0:1], mv[:, 0:1], float(L), None, Alu.mult)
            nc.vector.tensor_scalar(smm[t][:, 1:2], mv[:, 1:2], sq[:, 0:1], float(L), Alu.add, Alu.mult)
        for t in range(CT):
            nc.tensor.matmul(ps_g[:], sel_sb[t][:], smm[t][:], start=(t == 0), stop=(t == CT - 1))
        grp = pb.tile([G, 2], f32, tag="grp", name="grp")      # [mean_g, rstd_g]
        ex2 = pb.tile([G, 1], f32, tag="ex2", name="ex2")
        nc.vector.tensor_scalar(grp[:, 0:1], ps_g[:, 0:1], icnt_sb[:, 0:1], None, Alu.mult)
        nc.vector.tensor_scalar(ex2[:], ps_g[:, 1:2], icnt_sb[:, 0:1], None, Alu.mult)
        mm2 = pb.tile([G, 1], f32, tag="mm2", name="mm2")
        nc.vector.tensor_mul(mm2[:], grp[:, 0:1], grp[:, 0:1])
        var = pb.tile([G, 1], f32, tag="var", name="var")
        nc.vector.tensor_tensor(var[:], ex2[:], mm2[:], Alu.subtract)
        sd = pb.tile([G, 1], f32, tag="sd", name="sd")
        eps_sb = pb.tile([G, 1], f32, tag="eps", name="eps")
        nc.vector.memset(eps_sb[:], EPS)
        nc.scalar.activation(sd[:], var[:], Act.Sqrt, bias=eps_sb[:], scale=1.0)
        nc.vector.reciprocal(grp[:, 1:2], sd[:])

        xnt_sb = [pb.tile([128, L], bf16, tag=f"xnT{t}", name=f"xnT{t}") for t in range(CT)]
        rg_sb = pb.tile([128, CT], f32, tag="rg", name="rg")
        chst = [pb.tile([128, 2], f32, tag=f"chst{t}", name=f"chst{t}") for t in range(CT)]
        for t in range(CT):
            ps_b = ps.tile([128, 2], f32, tag="avB", name="avB", bufs=2)
            nc.tensor.matmul(ps_b[:], selT_sb[:, 128 * t:128 * (t + 1)], grp[:], start=True, stop=True)
            nc.vector.tensor_copy(chst[t][:], ps_b[:])
            nc.vector.tensor_mul(rg_sb[:, t:t + 1], chst[t][:, 1:2], gam_sb[:, t:t + 1])
            nc.vector.tensor_scalar(xnt_sb[t][:], xmt_sb[t][:], chst[t][:, 0:1], rg_sb[:, t:t + 1],
                                    Alu.subtract, Alu.mult)

        # ---- Phase 2: QKV ----
        qkT_sb = [pb.tile([128, L], bf16, tag=f"qkT{ot}", name=f"qkT{ot}") for ot in range(8)]
        for ot in range(8):
            for qc in range(QC):
                pq = ps.tile([128, 512], f32, tag=("avA" if (ot * QC + qc) % 2 == 0 else "avB"), name="pq", bufs=2)
                for ct in range(CT):
                    nc.tensor.matmul(pq[:], wqk_sb[ct][ot][:],
                                     xnt_sb[ct][:, 512 * qc:512 * (qc + 1)],
                                     start=(ct == 0), stop=(ct == CT - 1))
                nc.vector.tensor_scalar(qkT_sb[ot][:, 512 * qc:512 * (qc + 1)], pq[:],
                                        bqk_sb[:, ot:ot + 1], None, Alu.add)
        v_sb = [pb.tile([128, H, DH + 1], bf16, tag=f"v{kt}", name=f"v{kt}") for kt in range(NT)]
        for kt in range(NT):
            pv = ps.tile([128, 512], f32, tag=("avA" if kt % 2 == 0 else "avB"), name="pv", bufs=2)
            for ct in range(CT):
                nc.tensor.matmul(pv[:], xnt_sb[ct][:, 128 * kt:128 * (kt + 1)], wv_sb[ct][:],
                                 start=(ct == 0), stop=(ct == CT - 1))
            nc.vector.tensor_scalar(v_sb[kt][:, :, 0:DH],
                                    pv[:].rearrange("p (h d) -> p h d", h=H),
                                    vmask_sb[:, kt:kt + 1], None, Alu.mult)
            nc.vector.tensor_copy(v_sb[kt][:, :, DH], vmask_sb[:, kt:kt + 1].to_broadcast((128, H)))

        # ---- Phase 3: attention (head pairs packed into the PE array) ----
        attnT_sb = [pb.tile([128, L], bf16, tag=f"attnT{p}", name=f"attnT{p}") for p in range(CT)]
        bcast_sb = [pb.tile([64, 512], f32, tag=f"bcast{j}", name=f"bcast{j}") for j in range(2)]
        # expT2[j, kt, q]: exp'd transposed scores for the two heads of a pair
        expT2 = pb.tile([128, 2, NT, 512], bf16, tag="expT2", name="expT2")

        for p in range(CT):  # head pair p: heads 2p, 2p+1
            kT = qkT_sb[4 + p]
            qT = qkT_sb[p]
            for qc in range(QC):
                qs = slice(512 * qc, 512 * (qc + 1))
                avs = []
                for j in range(2):
                    av = ps.tile([128, 512], f32, tag=("avA" if j == 0 else "avB"),
                                 name=f"av{j}", bufs=2)
                    avs.append(av)
                def av_group(g):
                    for j in range(2):
                        h = 2 * p + j
                        for u in range(2):
                            kt = 2 * g + u
                            nc.tensor.matmul(avs[j][0:DH + 1, :], v_sb[kt][:, h, :],
                                             expT2[:, j, kt, :],
                                             start=(kt == 0), stop=(kt == NT - 1))
                for g in range(NT // 2):
                    # adjacent matmuls to rows 0-63 / 64-127 run concurrently in the array
                    s01 = ps.tile([128, 4, 512], f32, tag="s01", name="s01")
                    for u in range(2):
                        kt = 2 * g + u
                        ks = slice(128 * kt, 128 * (kt + 1))
                        nc.tensor.matmul(s01[:, u, :], kT[0:64, ks], qT[0:64, qs],
                                         start=True, stop=True)
                        nc.tensor.matmul(s01[:, 2 + u, :], kT[64:128, ks], qT[64:128, qs],
                                         start=True, stop=True)
                    if g > 0:
                        av_group(g - 1)   # PE fills while exp(g) runs on ScalarE
                    nc.scalar.activation(expT2[:, :, 2 * g:2 * g + 2, :], s01[:],
                                         Act.Exp, bias=0.0, scale=0.125)
                av_group(NT // 2 - 1)
                for j in range(2):
                    av = avs[j]
                    dn = pb.tile([1, 512], f32, tag=f"dn{j}", name=f"dn{j}", bufs=2)
                    nc.vector.tensor_tensor(dn[:], av[DH:DH + 1, :], qmv_sb[0:1, qs], Alu.mult)
                    rec = pb.tile([1, 512], f32, tag=f"rec{j}", name=f"rec{j}", bufs=2)
                    nc.vector.reciprocal_approx_fast(rec[:], dn[:])
                    nc.gpsimd.partition_broadcast(bcast_sb[j][:], rec[:])
                    rows = slice(64 * j, 64 * (j + 1))
                    nc.vector.tensor_tensor(attnT_sb[p][rows, qs], av[0:DH, :],
                                            bcast_sb[j][:], Alu.mult)

        xm_sb = [pb.tile([128, C], f32, tag=f"xm{t}", name=f"xm{t}") for t in range(NT)]
        for t in range(NT):
            nc.sync.dma_start(xm_sb[t][:], xm_d[128 * t:128 * (t + 1), :])

        # ---- Phase 4: projection + residual ----
        for qt in range(NT):
            po = ps.tile([128, 512], f32, tag=("avA" if qt % 2 == 0 else "avB"), name="po", bufs=2)
            for p in range(CT):
                nc.tensor.matmul(po[:], attnT_sb[p][:, 128 * qt:128 * (qt + 1)], wp_sb[p][:],
                                 start=(p == 0), stop=(p == CT - 1))
            o_sb = pb.tile([128, C], f32, tag=f"o{qt % 2}", name=f"o{qt % 2}")
            nc.vector.tensor_add(o_sb[:], po[:], xm_sb[qt][:])
            nc.sync.dma_start(out_d[128 * qt:128 * (qt + 1), :], o_sb[:])

    nc.compile()
    return nc


def _get_nc():
    if "nc" not in _CACHE:
        _CACHE["nc"] = _build()
    return _CACHE["nc"]


def _prep_weights(gamma, beta, Wqkv, bqkv, Wproj, bproj):
    """Host-side constant prep shared across cores."""
    W = np.asarray(Wqkv, np.float32)
    bq = np.asarray(bqkv, np.float32) + np.asarray(beta, np.float32) @ W   # fold beta
    Wp = np.asarray(Wproj, np.float32)
    bv = bq[2 * C:3 * C]
    # residual-side constant: bproj + bv @ Wproj (added to masked rows on host)
    resid_bias = np.asarray(bproj, np.float32) + bv @ Wp

    wqk = np.zeros((CT, 8, 128, 128), BF16)
    for ct in range(CT):
        for ot in range(8):
            wqk[ct, ot] = W[128 * ct:128 * (ct + 1), 128 * ot:128 * (ot + 1)].astype(BF16)
    wv = np.zeros((CT, 128, C), BF16)
    for ct in range(CT):
        wv[ct] = W[128 * ct:128 * (ct + 1), 2 * C:3 * C].astype(BF16)
    wp = np.zeros((CT, 128, C), BF16)
    for ct in range(CT):
        wp[ct] = Wp[128 * ct:128 * (ct + 1), :].astype(BF16)
    bqk = np.zeros((128, 8), np.float32)
    for ot in range(8):
        bqk[:, ot] = bq[128 * ot:128 * (ot + 1)]
    gam = np.asarray(gamma, np.float32).reshape(CT, 128).T.copy()
    sel = np.zeros((CT, 128, G), np.float32)
    for ct in range(CT):
        for c in range(128):
            sel[ct, c, (128 * ct + c) // CPG] = 1.0
    selT = np.zeros((G, C), np.float32)
    for c in range(C):
        selT[c // CPG, c] = 1.0
    return dict(wqk=wqk, wv=wv, wp=wp, bqk=bqk, gam=gam, sel=sel,
                selT=selT), resid_bias


def kernel(x, lengths, gamma, beta, Wqkv, bqkv, Wproj, bproj):
    from concourse.bass_utils import run_bass_kernel_spmd

    x = np.asarray(x, np.float32)
    lengths = np.asarray(lengths).astype(np.int64)
    const, resid_bias = _prep_weights(gamma, beta, Wqkv, bqkv, Wproj, bproj)

    in_maps = []
    for s in range(B):
        ln = int(lengths[s])
        mf = (np.arange(L) < ln).astype(np.float32)
        xm = x[s] * mf[:, None]
        xmr = xm + mf[:, None] * resid_bias[None, :]
        xmT = np.ascontiguousarray(xm.T)
        vmask = mf.reshape(NT, 128).T.copy()
        qmv = np.where(mf > 0, np.float32(1.0), np.float32(1e30)).reshape(1, L)
        icnt = np.full((G, 1), 1.0 / max(ln * CPG, 1), np.float32)
        m = dict(const)
        m.update(xmT=xmT, xmr=xmr, vmask=vmask, qmv=qmv, icnt=icnt)
        in_maps.append(m)

    nc = _get_nc()
    res = run_bass_kernel_spmd(nc, in_maps, core_ids=list(range(B)))
    _CACHE["last_res"] = res
    out = np.stack([res.results[s]["out"] for s in range(B)], axis=0)
    return out.astype(np.float32)


if __name__ == "__main__":
    rng = np.random.default_rng(0)
    x = rng.standard_normal((B, L, C), dtype=np.float32)
    lengths = rng.integers(L // 2, L + 1, size=(B,))
    gamma = np.ones(C, np.float32)
    beta = np.zeros(C, np.float32)
    Wqkv = (rng.standard_normal((C, 3 * C)) * 0.02).astype(np.float32)
    bqkv = np.zeros(3 * C, np.float32)
    Wproj = (rng.standard_normal((C, C)) * 0.02).astype(np.float32)
    bproj = np.zeros(C, np.float32)
    out = kernel(x=x, lengths=lengths, gamma=gamma, beta=beta, Wqkv=Wqkv,
                 bqkv=bqkv, Wproj=Wproj, bproj=bproj)
    print("out", out.shape, out.dtype, np.abs(out).max())

